# revision 2
# baseline (speedup 1.0000x reference)
"""Trainium2 Bass kernel for nn_BinarizeLayer.

out[b, f] = (medians[f] > 0) AND (inputs[b, f] >= medians[f])

Host preprocessing folds the two conditions into one comparison AND
prunes dead features: for the ~50% of features with medians[f] <= 0 the
output is False regardless of the input, so those input columns are
never shipped to (or read by) the device at all. The host gathers the
Fs = #(medians > 0) "live" columns into a compact [B, FP] array
(FP = Fs rounded up to a multiple of 4), the device computes
out = x_gathered >= medians_gathered, and the host scatters the packed
result back into the full [B, 8192] output (False elsewhere). This
halves HBM read traffic, which is the roofline for this kernel.

Data-parallel over 8 NeuronCores, processed as chunks of 32 consecutive
rows (~0.5 MiB, fully contiguous in DRAM). The load's access pattern
fans each chunk onto 128 partitions: partition p holds quarter-row
(row 32i + p//4, cols (p%4)*CQ ..), so HBM reads stay sequential
while compute still uses all 128 lanes.

UNEVEN SHARDING: the chip's HBM arbiter "parks" cores at ~300 GB/s
(vs the ~405 GB/s duty-cycle ceiling) for long stretches. Across 15
profiled runs the parking is strongly slot-dependent: EVEN logical
slots (= even physical NeuronCores) account for ~23 hard-parks vs ~3
for odd slots, with slot 6 the worst. Allocation is shaded
accordingly: core 6 gets 59 chunks, other even cores 63, odd cores 66
(sum 512 = full batch). One SPMD program handles this with per-core
branches: chunks 59-62 run under If(pid != 6), chunks 63-65 under
If(pid % 2 == 1).

Per chunk: DVE compares against a median tile host-prepared in the same
per-partition layout, emitting 0/1 bf16 bits; the tensor engine
bit-packs 8 partitions per byte with one constant [128,16] matmul
weight (2^(p%8) block-diagonal), accumulating exact small integers in
PSUM; the scalar engine evacuates PSUM to SBUF with an f32->u8 cast.
Each core stores ~1 MiB instead of ~8 MiB; the host unpacks bits and
inverts the partition bijection with pure reshapes.

Tail: EVERY core's last chunk (65 for odd cores, 62 for even cores
0/2/4, 58 for core 6 -- selected with If/Else on the partition id) is
loaded whole but compared/packed in <=512-column slabs; its PSUM slabs
are evacuated on the vector engine and stored via the sync ring, so
the post-load drain is short and runs in parallel with the scalar
engine's last stores.
"""

import numpy as np

import concourse.bacc as bacc
import concourse.mybir as mybir
from concourse import tile
from concourse.bass_utils import run_bass_kernel_spmd

N_CORES = 8
B, F = 16384, 8192
P = 128  # SBUF partitions
R = 32  # rows per chunk
C = P // R  # column quarters per row (4)
G = P // 8  # packed bytes' groups per chunk (16)
MM_N = 512  # matmul free-dim chunk (one PSUM bank)

# Chunks per core (sum must be B // R = 512); see module docstring.
CNT = [63, 66, 63, 66, 63, 66, 59, 66]
MID = 63  # even slots (except 6) stop here
BASE = min(CNT)  # unconditional chunk count (59)
MAXC = max(CNT)  # program-emitted chunk count (66)
STARTS = np.concatenate([[0], np.cumsum(CNT)]) * R  # global row offsets
XROWS = MAXC * R  # padded per-core input rows (2112)


def _build(fp):
    """Build the SPMD program for FP = fp gathered (padded) columns."""
    cq = fp // C  # columns per partition quarter-row
    nc = bacc.Bacc(
        "TRN2",
        target_bir_lowering=False,
        debug=False,
        num_devices=N_CORES,
    )
    x = nc.declare_dram_parameter("x", [XROWS, fp], mybir.dt.float32, isOutput=False)
    med = nc.declare_dram_parameter(
        "med", [P // 4, cq], mybir.dt.float32, isOutput=False
    )
    pw = nc.declare_dram_parameter("pw", [P, G], mybir.dt.float32, isOutput=False)
    out = nc.declare_dram_parameter(
        "out", [MAXC * G, cq], mybir.dt.uint8, isOutput=True
    )
    xv = x.rearrange("(i r) (c j) -> i (r c) j", r=R, c=C)

    # <=512-wide matmul slabs covering cq (last one may be partial).
    slabs = [(s, min(MM_N, cq - s)) for s in range(0, cq, MM_N)]

    with tile.TileContext(nc) as tc:
        with (
            tc.tile_pool(name="const", bufs=1) as cpool,
            tc.tile_pool(name="xp", bufs=12) as xpool,
            tc.tile_pool(name="bp", bufs=5) as bpool,
            tc.tile_pool(name="op", bufs=5) as opool,
            tc.tile_pool(name="ps", bufs=2, space="PSUM") as pspool,
        ):
            # Constants at the head of the scalar ring; the sync ring is
            # purely x-loads from instruction 0. The median tile's
            # layout is periodic every 4 partitions; a 32-partition
            # replica is loaded and doubled twice on the DVE (partition
            # bases must be 32-aligned) -- saves HBM during the ramp,
            # when the chip is most oversubscribed.
            med_t = cpool.tile([P, cq], mybir.dt.float32)
            nc.scalar.dma_start(out=med_t[0 : P // 4, :], in_=med[:])
            nc.vector.tensor_copy(
                out=med_t[P // 4 : P // 2, :], in_=med_t[0 : P // 4, :]
            )
            nc.vector.tensor_copy(
                out=med_t[P // 2 : P, :], in_=med_t[0 : P // 2, :]
            )
            pw_f32 = cpool.tile([P, G], mybir.dt.float32)
            pw_t = cpool.tile([P, G], mybir.dt.bfloat16)
            nc.scalar.dma_start(out=pw_f32[:], in_=pw[:])
            nc.vector.tensor_copy(out=pw_t[:], in_=pw_f32[:])

            pid = nc.partition_id()

            # Uniform per-chunk emission: grouped/batched program
            # structure makes the tile scheduler batch the sync-ring
            # dispatch waits, which starves the DMA ring in bursts.
            def body(i):
                xt = xpool.tile([P, cq], mybir.dt.float32, tag="x")
                nc.sync.dma_start(out=xt[:], in_=xv[i][:])
                bt = bpool.tile([P, cq], mybir.dt.bfloat16, tag="b")
                nc.vector.tensor_tensor(
                    bt[:], xt[:], med_t[:], mybir.AluOpType.is_ge
                )
                ps = pspool.tile([G, cq], mybir.dt.float32, tag="ps")
                for s, w in slabs:
                    nc.tensor.matmul(
                        ps[:, s : s + w],
                        pw_t[:],
                        bt[:, s : s + w],
                        start=True,
                        stop=True,
                    )
                pk = opool.tile([G, cq], mybir.dt.uint8, tag="o")
                nc.scalar.copy(out=pk[:], in_=ps[:])
                nc.scalar.dma_start(
                    out=out[i * G : (i + 1) * G, :], in_=pk[:]
                )

            def slab_tail(i):
                # Load whole, drain in <=512-col slabs on otherwise-idle
                # engines (DVE evac, sync-ring stores).
                xt = xpool.tile([P, cq], mybir.dt.float32, tag="x")
                nc.sync.dma_start(out=xt[:], in_=xv[i][:])
                ps = pspool.tile([G, cq], mybir.dt.float32, tag="ps")
                for s, w in slabs:
                    bt = bpool.tile([P, w], mybir.dt.bfloat16, tag="b")
                    nc.vector.tensor_tensor(
                        bt[:],
                        xt[:, s : s + w],
                        med_t[:, s : s + w],
                        mybir.AluOpType.is_ge,
                    )
                    nc.tensor.matmul(
                        ps[:, s : s + w], pw_t[:], bt[:], start=True, stop=True
                    )
                    pk = opool.tile([G, w], mybir.dt.uint8, tag="ot")
                    nc.vector.tensor_copy(out=pk[:], in_=ps[:, s : s + w])
                    nc.sync.dma_start(
                        out=out[i * G : (i + 1) * G, s : s + w], in_=pk[:]
                    )

            # Every core's LAST chunk gets the slab_tail drain (the
            # regular body ends with a scalar-serialized drain;
            # slab_tail drains on otherwise-idle engines).
            for i in range(BASE - 1):
                body(i)
            with tc.If(pid != 6) as c6:
                body(BASE - 1)
                for i in range(BASE, MID - 1):
                    body(i)
                with tc.If(pid % 2 == 1) as codd:
                    body(MID - 1)
                    for i in range(MID, MAXC - 1):
                        body(i)
                    slab_tail(MAXC - 1)
                with codd.Else():
                    slab_tail(MID - 1)
            with c6.Else():
                slab_tail(BASE - 1)
    nc.compile()
    return nc


def _pack_weights():
    pw = np.zeros((P, G), dtype=np.float32)
    for p in range(P):
        pw[p, p // 8] = float(1 << (p % 8))
    return pw


def _select(medians):
    """Live-feature index set and padded width FP (multiple of 4)."""
    m = np.asarray(medians, dtype=np.float32)
    idx = np.flatnonzero(m > 0)
    fs = int(idx.size)
    fp = -(-max(fs, 4) // 4) * 4
    return m, idx, fs, fp


def _in_maps(inputs, medians):
    x = np.asarray(inputs, dtype=np.float32)
    m, idx, fs, fp = _select(medians)
    cq = fp // C
    # Gathered medians, padded with +inf (pad columns compare False).
    m2 = np.full(fp, np.inf, dtype=np.float32)
    m2[:fs] = m[idx]
    med = np.ascontiguousarray(
        np.broadcast_to(m2.reshape(1, C, cq), (P // 4 // C, C, cq)).reshape(
            P // 4, cq
        )
    )
    pw = _pack_weights()
    xg = x[:, idx]  # [B, fs] gathered live columns
    maps = []
    for c in range(N_CORES):
        xc = np.zeros((XROWS, fp), dtype=np.float32)
        rows = CNT[c] * R
        xc[:rows, :fs] = xg[STARTS[c] : STARTS[c] + rows]
        maps.append({"x": xc, "med": med, "pw": pw})
    return maps


def _decode(packed, cnt, fp):
    """[MAXC*G, cq] u8 -> [cnt*R, fp] 0/1 rows for one core."""
    cq = fp // C
    a = packed.reshape(MAXC, G, 1, cq)
    bits = np.unpackbits(a, axis=2, bitorder="little")  # [i, g, k, j]
    # partition p = 8g + k -> (row p//4, quarter p%4)
    bits = bits.reshape(MAXC, P, cq).reshape(MAXC, R, C, cq)
    return bits.reshape(XROWS, fp)[: cnt * R]


def kernel(inputs, medians):
    m, idx, fs, fp = _select(medians)
    if fs == 0:
        return np.zeros((np.asarray(inputs).shape[0], m.size), dtype=bool)
    in_maps = _in_maps(inputs, medians)
    last_err = None
    for _ in range(3):  # transient axon/NRT failures happen; retry
        try:
            nc = _build(fp)
            res = run_bass_kernel_spmd(nc, in_maps, list(range(N_CORES))).results
            break
        except Exception as e:  # noqa: BLE001
            last_err = e
    else:
        raise last_err
    gathered = np.concatenate(
        [_decode(r["out"], CNT[c], fp) for c, r in enumerate(res)], axis=0
    )
    out = np.zeros((gathered.shape[0], m.size), dtype=bool)
    out[:, idx] = gathered[:, :fs].astype(bool)
    return out
